# revision 4
# baseline (speedup 1.0000x reference)
"""Trainium2 Bass kernel for a 2-layer Realformer-style cross-attention
transformer (B=8, S=1024, D=512, H=8, DFF=2048), data-parallel over batch
across 8 NeuronCores (one batch element per core, no collectives).

Layout strategy: activations are kept feature-major ("transposed", [D, S]).
Per-head Q/K are stored as 128-partition "cat" tiles holding BOTH layers'
64-dim head projections (parity-arranged), so layer-1 Realformer scores
(q1.k1 + q0.k0) are a single K=128 matmul; layer-0 scores run as
row-tiled concurrent pairs (tile_position) using only 64 PE rows each.
Softmax denominators come for free from a ones column appended to V-heads.
LayerNorm stats (sum/sumsq) run as col-tiled concurrent M=1 matmuls.
Output is written feature-major [D, S] and transposed on the host.
"""

import sys

sys.path.insert(0, "/opt/trn_rl_repo")

import numpy as np
import ml_dtypes

B, S, D, H, HD, DFF, L = 8, 1024, 512, 8, 64, 2048, 2
P = 128
DC = D // P            # 4 d-chunks
FC = DFF // P          # 16 f-chunks
ST = S // P            # 8 seq tiles
NSQ = 2                # sq halves of 512
SQW = S // NSQ         # 512
KP = ST // 2           # 4 key-tile pairs
EPS = 1e-5
N_CORES = 8

BF16 = ml_dtypes.bfloat16

_CACHE = {}


def _build_nc(repeats=1, ablate=(), mmn=512):
    import concourse.bacc as bacc
    import concourse.tile as tile
    from concourse import mybir

    f32 = mybir.dt.float32
    bf16 = mybir.dt.bfloat16
    AF = mybir.ActivationFunctionType
    OP = mybir.AluOpType

    nc = bacc.Bacc(None, target_bir_lowering=False)

    # ---- external params ----
    xT_d = nc.declare_dram_parameter("xT", [D, S], bf16, isOutput=False)
    kT_d = nc.declare_dram_parameter("kT", [D, S], bf16, isOutput=False)
    vT_d = nc.declare_dram_parameter("vT", [D, S], bf16, isOutput=False)
    wq_d, wk_d, wv_d, wo_d, wf1_d, wf2_d = [], [], [], [], [], []
    bq_d, bk_d, bo_d, bf1_d, bf2_d = [], [], [], [], []
    g1_d, b1_d, g2_d, b2_d = [], [], [], []
    for i in range(L):
        wq_d.append(nc.declare_dram_parameter(f"wq{i}", [D, D], bf16, isOutput=False))
        wk_d.append(nc.declare_dram_parameter(f"wk{i}", [D, D], bf16, isOutput=False))
        wv_d.append(nc.declare_dram_parameter(f"wv{i}", [D, D], bf16, isOutput=False))
        wo_d.append(nc.declare_dram_parameter(f"wo{i}", [D, D], bf16, isOutput=False))
        wf1_d.append(nc.declare_dram_parameter(f"wf1_{i}", [D, DFF], bf16, isOutput=False))
        wf2_d.append(nc.declare_dram_parameter(f"wf2_{i}", [DFF, D], bf16, isOutput=False))
        bq_d.append(nc.declare_dram_parameter(f"bq{i}", [D], f32, isOutput=False))
        bk_d.append(nc.declare_dram_parameter(f"bk{i}", [D], f32, isOutput=False))
        bo_d.append(nc.declare_dram_parameter(f"bo{i}", [D], f32, isOutput=False))
        bf1_d.append(nc.declare_dram_parameter(f"bf1_{i}", [DFF], f32, isOutput=False))
        bf2_d.append(nc.declare_dram_parameter(f"bf2_{i}", [D], f32, isOutput=False))
        g1_d.append(nc.declare_dram_parameter(f"g1_{i}", [D], f32, isOutput=False))
        b1_d.append(nc.declare_dram_parameter(f"b1_{i}", [D], f32, isOutput=False))
        g2_d.append(nc.declare_dram_parameter(f"g2_{i}", [D], f32, isOutput=False))
        b2_d.append(nc.declare_dram_parameter(f"b2_{i}", [D], f32, isOutput=False))
    y_d = nc.declare_dram_parameter("y", [D, S], f32, isOutput=True)

    with tile.TileContext(nc) as tc:
        import contextlib

        ctx = contextlib.ExitStack()
        with ctx:
            const = ctx.enter_context(tc.tile_pool(name="const", bufs=1))
            wpool = ctx.enter_context(tc.tile_pool(name="wpool", bufs=1))
            catp = ctx.enter_context(tc.tile_pool(name="catp", bufs=1))
            stream = ctx.enter_context(tc.tile_pool(name="stream", bufs=6))
            vhp = ctx.enter_context(tc.tile_pool(name="vhp", bufs=8))
            expp = ctx.enter_context(tc.tile_pool(name="expp", bufs=8))
            outp = ctx.enter_context(tc.tile_pool(name="outp", bufs=4))
            htp = ctx.enter_context(tc.tile_pool(name="htp", bufs=16))
            xsqp = ctx.enter_context(tc.tile_pool(name="xsqp", bufs=2))
            rows = ctx.enter_context(tc.tile_pool(name="rows", bufs=5))
            bcp = ctx.enter_context(tc.tile_pool(name="bcp", bufs=2))
            tmpp = ctx.enter_context(tc.tile_pool(name="tmpp", bufs=2))
            pp = ctx.enter_context(tc.tile_pool(name="pp", bufs=2, space="PSUM"))
            scp = ctx.enter_context(tc.tile_pool(name="scp", bufs=2, space="PSUM"))
            avp = ctx.enter_context(tc.tile_pool(name="avp", bufs=2, space="PSUM"))

            # ---- constants ----
            ones_bf = const.tile([P, 1], bf16, tag="ones_bf")
            nc.vector.memset(ones_bf, 1.0)
            eps_t = const.tile([1, 1], f32, tag="eps")
            nc.vector.memset(eps_t, EPS)

            def load_cols(dram, n):
                # [n*P] dram vector -> [P, n] sbuf tile, col c = v[c*P:(c+1)*P]
                t = const.tile([P, n], f32, tag=f"cols{dram.name}")
                nc.sync.dma_start(out=t, in_=dram[:].rearrange("(c p) -> p c", p=P))
                return t

            bq_t = [load_cols(bq_d[i], DC) for i in range(L)]
            bk_t = [load_cols(bk_d[i], DC) for i in range(L)]
            bo_t = [load_cols(bo_d[i], DC) for i in range(L)]
            bf1_t = [load_cols(bf1_d[i], FC) for i in range(L)]
            bf2_t = [load_cols(bf2_d[i], DC) for i in range(L)]
            g1_t = [load_cols(g1_d[i], DC) for i in range(L)]
            b1_t = [load_cols(b1_d[i], DC) for i in range(L)]
            g2_t = [load_cols(g2_d[i], DC) for i in range(L)]
            b2_t = [load_cols(b2_d[i], DC) for i in range(L)]

            def load_w(dram, nchunk, ncols, tag):
                # [nchunk*P, ncols] dram -> [P, nchunk, ncols] sbuf
                t = wpool.tile([P, nchunk, ncols], bf16, tag=tag)
                nc.sync.dma_start(
                    out=t, in_=dram[:].rearrange("(c p) e -> p c e", p=P)
                )
                return t

            def load_fm(dram, tag, pool):
                # [D, S] dram -> list of DC tiles [P, S]
                ts = []
                for c in range(DC):
                    t = pool.tile([P, S], bf16, tag=f"{tag}{c}")
                    nc.sync.dma_start(out=t, in_=dram[c * P : (c + 1) * P, :])
                    ts.append(t)
                return ts

            xin = load_fm(xT_d, "xin", const)
            kt_t = load_fm(kT_d, "kin", const)
            vt_t = load_fm(vT_d, "vin", const)

            def build_body():
                # per-head cat tiles: [128, S]; within each, head h's layer-li
                # 64-dim projection lives at partition base:
                #   base(h, li) = 0 if (h+li) even else 64
                qcat = [catp.tile([P, S], bf16, tag=f"qcat{h}", name=f"qcat{h}")
                        for h in range(H)]
                kcat = [catp.tile([P, S], bf16, tag=f"kcat{h}", name=f"kcat{h}")
                        for h in range(H)]

                xcur = xin
                for li in range(L):
                    wq_t = load_w(wq_d[li], DC, D, tag="wq")
                    wk_t = load_w(wk_d[li], DC, D, tag="wk")
                    wv_t = load_w(wv_d[li], DC, D, tag="wv")
                    wo_t = load_w(wo_d[li], DC, D, tag="wo")
                    wf1_t = load_w(wf1_d[li], DC, DFF, tag="wf1")
                    wf2_t = load_w(wf2_d[li], FC, D, tag="wf2")

                    def proj_qk(w_t, rhs_tiles, bias_t, cat, sq_sl, on_pool):
                        # project into cat[h][base(h,li):base+64, sq_sl]
                        # NOTE for li==1 the host supplies bias vectors with
                        # 64-halves swapped within each 128-chunk, so the
                        # scalar AP partition range matches the PSUM half.
                        for pt in range(DC):
                            hA, hB = 2 * pt, 2 * pt + 1
                            ps = pp.tile([P, SQW], f32, tag="pp")
                            if li == 0:
                                for dc in range(DC):
                                    nc.tensor.matmul(
                                        ps,
                                        w_t[:, dc, pt * P : (pt + 1) * P],
                                        rhs_tiles[dc][:, sq_sl],
                                        start=(dc == 0),
                                        stop=(dc == DC - 1),
                                    )
                                top_h, bot_h = hA, hB
                            else:
                                # col-tiled concurrent M=64 pair (shared rhs)
                                for dc in range(DC):
                                    nc.tensor.matmul(
                                        ps[64:128, :],
                                        w_t[:, dc, hA * HD : (hA + 1) * HD],
                                        rhs_tiles[dc][:, sq_sl],
                                        start=(dc == 0),
                                        stop=(dc == DC - 1),
                                        tile_position=(0, 64),
                                    )
                                    nc.tensor.matmul(
                                        ps[0:64, :],
                                        w_t[:, dc, hB * HD : (hB + 1) * HD],
                                        rhs_tiles[dc][:, sq_sl],
                                        start=(dc == 0),
                                        stop=(dc == DC - 1),
                                        tile_position=(0, 0),
                                    )
                                top_h, bot_h = hB, hA
                            eng = nc.vector  # gpsimd cannot read PSUM
                            eng.tensor_scalar(
                                cat[top_h][0:64, sq_sl],
                                ps[0:64, :],
                                bias_t[0:64, pt : pt + 1],
                                None,
                                OP.add,
                            )
                            eng.tensor_scalar(
                                cat[bot_h][64:128, sq_sl],
                                ps[64:128, :],
                                bias_t[64:128, pt : pt + 1],
                                None,
                                OP.add,
                            )

                    # ---- K projection (full S) on Pool for bias adds ----
                    for s2 in range(NSQ):
                        proj_qk(wk_t, kt_t, bk_t[li], kcat,
                                slice(s2 * SQW, (s2 + 1) * SQW), on_pool=True)

                    # ---- V-heads, seq-major with ones column: [P, H, HD+1] ----
                    vh_t = []
                    for st in range(ST):
                        ps = pp.tile([P, D], f32, tag="pp")
                        for dc in range(DC):
                            nc.tensor.matmul(
                                ps,
                                vt_t[dc][:, st * P : (st + 1) * P],
                                wv_t[:, dc, :],
                                start=(dc == 0),
                                stop=(dc == DC - 1),
                            )
                        t = vhp.tile([P, H, HD + 1], bf16, tag="vh")
                        nc.gpsimd.memset(t[:, :, HD : HD + 1], 1.0)
                        nc.vector.tensor_copy(
                            out=t[:, :, 0:HD], in_=ps[:].rearrange("p (h w) -> p h w", h=H)
                        )
                        vh_t.append(t)

                    def attn_head_phase1(h, sq_sl):
                        # scores + exp for one head (layer-1 path, K=128 cat)
                        ex_tiles = []
                        for kp in range(KP):
                            sc = scp.tile([P, 2, SQW], f32, tag="sc")
                            for j in range(2):
                                kt = 2 * kp + j
                                nc.tensor.matmul(
                                    sc[:, j, :],
                                    kcat[h][:, kt * P : (kt + 1) * P],
                                    qcat[h][:, sq_sl],
                                    start=True,
                                    stop=True,
                                )
                            ex = expp.tile([P, 2, SQW], bf16, tag="exp",
                                           name=f"ex{h}_{kp}")
                            nc.scalar.activation(ex, sc, AF.Exp)
                            ex_tiles.append(ex)
                        return ex_tiles

                    def attn_pair_phase1(pt, sq_sl):
                        # layer-0: row-tiled concurrent score pairs for
                        # heads (2pt, 2pt+1); K=64 each.
                        hA, hB = 2 * pt, 2 * pt + 1
                        exA, exB = [], []
                        for kp in range(KP):
                            scA = scp.tile([P, 2, SQW], f32, tag="sc")
                            scB = scp.tile([P, 2, SQW], f32, tag="sc")
                            for j in range(2):
                                kt = 2 * kp + j
                                ksl = slice(kt * P, (kt + 1) * P)
                                nc.tensor.matmul(
                                    scA[:, j, :],
                                    kcat[hA][0:64, ksl],
                                    qcat[hA][0:64, sq_sl],
                                    start=True,
                                    stop=True,
                                    tile_position=(0, 0),
                                )
                                nc.tensor.matmul(
                                    scB[:, j, :],
                                    kcat[hB][64:128, ksl],
                                    qcat[hB][64:128, sq_sl],
                                    start=True,
                                    stop=True,
                                    tile_position=(64, 0),
                                )
                            eA = expp.tile([P, 2, SQW], bf16, tag="exp",
                                           name=f"exA{pt}_{kp}")
                            nc.scalar.activation(eA, scA, AF.Exp)
                            exA.append(eA)
                            eB = expp.tile([P, 2, SQW], bf16, tag="exp",
                                           name=f"exB{pt}_{kp}")
                            nc.scalar.activation(eB, scB, AF.Exp)
                            exB.append(eB)
                        return exA, exB

                    def attn_phase2(h, ex_tiles, outt, sq_sl):
                        # AV accumulation + softmax normalize
                        pt_, pb_ = h // 2, (h % 2) * HD
                        av = avp.tile([HD + 1, SQW], f32, tag="av")
                        for kp in range(KP):
                            for j in range(2):
                                kt = 2 * kp + j
                                nc.tensor.matmul(
                                    av,
                                    vh_t[kt][:, h, :],
                                    ex_tiles[kp][:, j, :],
                                    start=(kt == 0),
                                    stop=(kt == ST - 1),
                                )
                        rec = rows.tile([1, SQW], f32, tag="rows")
                        nc.vector.reciprocal(rec, av[HD : HD + 1, :])
                        bc = bcp.tile([HD, SQW], f32, tag="bc64")
                        nc.gpsimd.partition_broadcast(bc, rec)
                        nc.vector.tensor_mul(
                            outt[pt_][pb_ : pb_ + HD, sq_sl], av[0:HD, :], bc
                        )

                    def layernorm_half(x_in, g_t, b_t, sq_sl, out_f32):
                        # returns list of DC output tiles ([P,SQW] views)
                        xsq = []
                        for dc in range(DC):
                            t = xsqp.tile([P, SQW], bf16, tag="xsq")
                            nc.vector.tensor_mul(t, x_in[dc][:, sq_sl],
                                                 x_in[dc][:, sq_sl])
                            xsq.append(t)
                        stat = pp.tile([33, SQW], f32, tag="pp")
                        for dc in range(DC):
                            nc.tensor.matmul(
                                stat[0:1, :], ones_bf, x_in[dc][:, sq_sl],
                                start=(dc == 0), stop=(dc == DC - 1),
                                tile_position=(0, 0),
                            )
                            nc.tensor.matmul(
                                stat[32:33, :], ones_bf, xsq[dc],
                                start=(dc == 0), stop=(dc == DC - 1),
                                tile_position=(0, 32),
                            )
                        mu = rows.tile([1, SQW], f32, tag="rows")
                        nc.vector.tensor_scalar(mu, stat[0:1, :], 1.0 / D, None,
                                                OP.mult)
                        msq = rows.tile([1, SQW], f32, tag="rows")
                        nc.vector.tensor_mul(msq, mu, mu)
                        var = rows.tile([1, SQW], f32, tag="rows")
                        nc.vector.scalar_tensor_tensor(
                            var, stat[32:33, :], 1.0 / D, msq, OP.mult, OP.subtract
                        )
                        std = rows.tile([1, SQW], f32, tag="rows")
                        nc.scalar.activation(std, var, AF.Sqrt, bias=eps_t[0:1, :])
                        a_row = rows.tile([1, SQW], f32, tag="rows")
                        nc.vector.reciprocal(a_row, std)
                        c_row = rows.tile([1, SQW], f32, tag="rows")
                        nc.vector.scalar_tensor_tensor(
                            c_row, mu, -1.0, a_row, OP.mult, OP.mult
                        )
                        a_bc = bcp.tile([P, SQW], f32, tag="bc128")
                        nc.gpsimd.partition_broadcast(a_bc, a_row)
                        c_bc = bcp.tile([P, SQW], f32, tag="bc128")
                        nc.gpsimd.partition_broadcast(c_bc, c_row)
                        outs = []
                        for dc in range(DC):
                            # cgb = c_bc*g + b  (per-dc, on Pool)
                            cgb = bcp.tile([P, SQW], f32, tag="cgb")
                            nc.gpsimd.tensor_scalar(
                                cgb, c_bc, g_t[:, dc : dc + 1],
                                b_t[:, dc : dc + 1], OP.mult, OP.add,
                            )
                            # t1 = (x*g) * a_bc
                            t1 = tmpp.tile([P, SQW], f32, tag="tmp")
                            nc.vector.scalar_tensor_tensor(
                                t1, x_in[dc][:, sq_sl], g_t[:, dc : dc + 1],
                                a_bc, OP.mult, OP.mult,
                            )
                            if out_f32:
                                o = tmpp.tile([P, SQW], f32, tag="fin", bufs=4)
                                nc.vector.tensor_add(o, t1, cgb)
                            else:
                                o = stream.tile([P, SQW], bf16, tag="stream",
                                                name=f"lnout{dc}")
                                nc.vector.tensor_add(o, t1, cgb)
                            outs.append(o)
                        return outs

                    # ---- per sq half: Q proj, attention, O, LN1, FFN, LN2 ----
                    xnext = ([stream.tile([P, S], bf16, tag="xbig", bufs=4,
                                          name=f"xnext{_i}") for _i in range(DC)]
                             if li < L - 1 else None)
                    for sqh in range(NSQ):
                        sq_sl = slice(sqh * SQW, (sqh + 1) * SQW)
                        proj_qk(wq_t, xcur, bq_t[li], qcat, sq_sl, on_pool=False)

                        outt = [outp.tile([P, SQW], bf16, tag="outt",
                                          name=f"outt{_i}") for _i in range(DC)]
                        if li == 0:
                            for pt in range(DC):
                                hA, hB = 2 * pt, 2 * pt + 1
                                exA, exB = attn_pair_phase1(pt, sq_sl)
                                attn_phase2(hA, exA, outt, slice(0, SQW))
                                attn_phase2(hB, exB, outt, slice(0, SQW))
                        else:
                            for h in range(H):
                                ex = attn_head_phase1(h, sq_sl)
                                attn_phase2(h, ex, outt, slice(0, SQW))

                        # O-projection + gated residual
                        xnew = []
                        for ft in range(DC):
                            ps = pp.tile([P, SQW], f32, tag="pp")
                            for ec in range(DC):
                                nc.tensor.matmul(
                                    ps,
                                    wo_t[:, ec, ft * P : (ft + 1) * P],
                                    outt[ec],
                                    start=(ec == 0),
                                    stop=(ec == DC - 1),
                                )
                            xn = stream.tile([P, SQW], bf16, tag="stream",
                                             name=f"xnew{ft}")
                            nc.vector.scalar_tensor_tensor(
                                xn, ps, bo_t[li][:, ft : ft + 1],
                                xcur[ft][:, sq_sl], OP.add, OP.add,
                            )
                            xnew.append(xn)

                        full = slice(0, SQW)
                        xln = layernorm_half(xnew, g1_t[li], b1_t[li], full,
                                             out_f32=False)

                        # ---- FFN ----
                        ht = []
                        for ft in range(FC):
                            ps = pp.tile([P, SQW], f32, tag="pp")
                            for dc in range(DC):
                                nc.tensor.matmul(
                                    ps,
                                    wf1_t[:, dc, ft * P : (ft + 1) * P],
                                    xln[dc],
                                    start=(dc == 0),
                                    stop=(dc == DC - 1),
                                )
                            t = htp.tile([P, SQW], bf16, tag="ht")
                            nc.scalar.activation(
                                t, ps, AF.Gelu, bias=bf1_t[li][:, ft : ft + 1]
                            )
                            ht.append(t)
                        x2 = []
                        for dt_ in range(DC):
                            ps = pp.tile([P, SQW], f32, tag="pp")
                            for fc in range(FC):
                                nc.tensor.matmul(
                                    ps,
                                    wf2_t[:, fc, dt_ * P : (dt_ + 1) * P],
                                    ht[fc],
                                    start=(fc == 0),
                                    stop=(fc == FC - 1),
                                )
                            xt = stream.tile([P, SQW], bf16, tag="stream",
                                             name=f"x2_{dt_}")
                            nc.vector.scalar_tensor_tensor(
                                xt, ps, bf2_t[li][:, dt_ : dt_ + 1],
                                xln[dt_], OP.add, OP.add,
                            )
                            x2.append(xt)

                        if li < L - 1:
                            fin = layernorm_half(x2, g2_t[li], b2_t[li], full,
                                                 out_f32=False)
                            for dc in range(DC):
                                nc.vector.tensor_copy(
                                    out=xnext[dc][:, sq_sl], in_=fin[dc]
                                )
                        else:
                            fin = layernorm_half(x2, g2_t[li], b2_t[li], full,
                                                 out_f32=True)
                            for dc in range(DC):
                                nc.sync.dma_start(
                                    out=y_d[dc * P : (dc + 1) * P, sq_sl],
                                    in_=fin[dc],
                                )
                    if li < L - 1:
                        xcur = xnext

            if repeats == 1:
                build_body()
            else:
                with tc.For_i(0, repeats, 1,
                              hint_engines=(mybir.EngineType.Pool,
                                            mybir.EngineType.Activation,
                                            mybir.EngineType.PE,
                                            mybir.EngineType.DVE,
                                            mybir.EngineType.SP)):
                    build_body()

    nc.compile()
    return nc


def _swap64(v):
    # swap 64-halves within each 128-chunk (matches the li=1 PSUM half layout)
    return np.ascontiguousarray(
        v.reshape(-1, 2, 64)[:, ::-1, :].reshape(v.shape)
    )


def _prep_inputs(inputs):
    """Host-side folding + sharding. Returns per-core in_maps."""
    f = {k: np.asarray(v, dtype=np.float32) for k, v in inputs.items()}
    q, k, v = f["q"], f["k"], f["v"]
    maps_common = {}
    for i in range(L):
        eff = f["scale"][i] * np.clip(f["extra_scale"][i], 0.01, 50.0)
        sp_a = np.log1p(np.exp(f["gate_attn"][i]))
        sp_f = np.log1p(np.exp(f["gate_ffn"][i]))
        wq = (f["WQ"][i] * eff).astype(BF16)
        wk = f["WK"][i].astype(BF16)
        wv = f["WV"][i].astype(BF16)
        wo = (f["WO"][i] * sp_a).astype(BF16)
        wf1 = f["Wf1"][i].astype(BF16)
        wf2 = (f["Wf2"][i] * sp_f).astype(BF16)
        bq = (f["bQ"][i] * eff).astype(np.float32)
        bk = f["bK"][i].astype(np.float32)
        if i == 1:
            bq = _swap64(bq)
            bk = _swap64(bk)
        # fold V bias through O projection: (out + bV) @ WO + bO
        bo = (sp_a * (f["bO"][i] + f["bV"][i] @ f["WO"][i])).astype(np.float32)
        bf1 = f["bf1"][i].astype(np.float32)
        bf2 = (f["bf2"][i] * sp_f).astype(np.float32)
        maps_common.update({
            f"wq{i}": wq, f"wk{i}": wk, f"wv{i}": wv, f"wo{i}": wo,
            f"wf1_{i}": wf1, f"wf2_{i}": wf2,
            f"bq{i}": bq, f"bk{i}": bk, f"bo{i}": bo,
            f"bf1_{i}": bf1, f"bf2_{i}": bf2,
            f"g1_{i}": f["ln1_g"][i].astype(np.float32),
            f"b1_{i}": f["ln1_b"][i].astype(np.float32),
            f"g2_{i}": f["ln2_g"][i].astype(np.float32),
            f"b2_{i}": f["ln2_b"][i].astype(np.float32),
        })
    in_maps = []
    for b in range(B):
        m = dict(maps_common)
        m["xT"] = np.ascontiguousarray(q[b].T).astype(BF16)
        m["kT"] = np.ascontiguousarray(k[b].T).astype(BF16)
        m["vT"] = np.ascontiguousarray(v[b].T).astype(BF16)
        in_maps.append(m)
    return in_maps


def get_nc(repeats=1, ablate=(), mmn=512):
    key = ("nc", repeats, tuple(ablate), mmn)
    if key not in _CACHE:
        _CACHE[key] = _build_nc(repeats, ablate=tuple(ablate), mmn=mmn)
    return _CACHE[key]


def kernel(**inputs) -> np.ndarray:
    from concourse.bass_utils import run_bass_kernel_spmd

    nc = get_nc()
    in_maps = _prep_inputs(inputs)
    res = run_bass_kernel_spmd(nc, in_maps, core_ids=list(range(N_CORES)))
    # y comes back feature-major [D, S]; transpose per batch element
    out = np.stack(
        [np.asarray(res.results[b]["y"]).T for b in range(B)], axis=0
    )
    return np.ascontiguousarray(out).astype(np.float32)
